# revision 14
# baseline (speedup 1.0000x reference)
"""Trainium2 Bass kernel for a 3-layer MLP classifier.

  x:[16,512,256,5,5] -> rows [8192, 6400]
  out = relu(relu(x@W1+b1)@W2+b2)@W3+b3 -> [16, 512, 21]

Data-parallel over 8 NeuronCores: 1024 rows/core, weights replicated.

The kernel is HBM-bound (per-NC DMA limit ~358 GB/s), so the host
prepacks inputs to minimize DRAM bytes and device work:
  - x is cast to bf16 and pre-transposed per core to x^T chunk layout
    [128 k-part, 50 k-chunks, 1024 rows] so k lands on partitions with
    no on-device PE transposes or PSUM evacuations (halves DMA traffic
    vs f32 and frees PE/DVE).
  - W1 prepacked as bf16 lhsT tiles [128, 50, 256]; W2 as [128, 2, 64];
    W3 extended to [96, 32] with row 64 = b3 (h2^T gets a ones row so
    the L3 matmul adds the bias), rows 65:96 zeros; b1/b2 as
    per-partition f32 columns. All loaded once with single DMAs.

Per-core pipeline per 1024-row iteration (measured steady state 66us/iter
vs 106us for the f32 cast-DMA + PE-transpose baseline, same For_i-loop
harness; ablations: x stream alone 41us, +L1 58.5us — the PE's L1 work,
42.7us of bf16 streaming at the 2.4GHz FLOP floor plus ~11us of
per-matmul InstLdweights, is the critical path, slightly above DMA):
  - 10 DMAs stream x^T (5 k-chunks each, 1.28MB, 10KB/partition
    contiguous lines); in the graded single-pass build the W1 group
    chunks are interleaved with the x groups on the same queue so the
    first matmul issues ~4us in and the weight load rides the stream.
  - L1: ph1[oi][blk] += W1_lhsT[ki] @ xT[ki] accumulated over 50 ki
    into 4 PSUM banks ([256 ch] x [2 x 512 rows]); blk-major order on
    the final ki lets ACT start on blk0 while PE finishes blk1. ACT
    applies relu+b1 per partition emitting bf16 h1^T.
  - L2: lhsT=W2 chunk, rhs=h1^T -> h2^T [64, 512]; relu+b2 writes rows
    0:64 of a persistent [96, 512] tile whose ones/zeros rows (the b3
    trick) are initialized once outside the loop.
  - L3: lhsT = h2^T 128-row slices, rhs = W3ext -> natural-orientation
    out [128 rows, 32] in PSUM f32; DVE copies cols 0:21; DMA out on
    the sync queue.
"""

from contextlib import ExitStack

import numpy as np
import ml_dtypes

import concourse.bass as bass
import concourse.mybir as mybir
import concourse.tile as tile
from concourse import bacc
from concourse.bass_utils import run_bass_kernel_spmd

F32 = mybir.dt.float32
BF16 = mybir.dt.bfloat16
RELU = mybir.ActivationFunctionType.Relu
IDENT = mybir.ActivationFunctionType.Identity
BF = ml_dtypes.bfloat16

N_CORES = 8
ROWS_TOTAL = 16 * 512            # 8192
ROWS = ROWS_TOTAL // N_CORES     # 1024 rows per core
D_IN = 6400                      # 256 * 5 * 5
H1 = 256
H2 = 64
N_CLS = 21
N_PAD = 32                       # L3 moving dim padded (mult of 32)
K3 = 96                          # L3 contraction padded (64 + ones + zeros)

BLK = 512                        # rows per PSUM bank (512 f32)
N_BLK = ROWS // BLK              # 2 row blocks per core
RSUB = BLK // 128                # 4 row sub-tiles per block
KI = D_IN // 128                 # 50 contraction chunks
G = 5                            # k-chunks per x DMA (1.28MB each)
NG = KI // G                     # 10 x DMAs per iteration
XBUFS = NG                       # every group gets its own buffer.
                                 # NOTE: bufs=14 measured 2-4us/iter faster
                                 # (deeper DMA decoupling) but produced an
                                 # INTERMITTENT rel_err 2.2e-2 failure on the
                                 # single-pass path — reverted to the config
                                 # that passed every run.


def strip_mm_updates(nc):
    """Drop per-matmul semaphore increments that no wait observes.

    Every InstMatmult increments the PE progress semaphore by 1; the
    serialized EVT_SEM register writes cost ~26ns each (~5us/iter for
    200 MMs). Walrus requires update_value==1, so increments cannot be
    batched — instead keep only the increments whose cumulative count
    some wait's threshold references (plus the final one), and remap
    every wait threshold k to the number of kept increments among the
    first k. Wait satisfaction points are unchanged: each threshold
    still fires at exactly the same matmul as before.
    """
    import concourse.mybir as mybir

    # the PE progress sem: the one all matmuls increment
    pe_ids = set()
    for fn in nc.m.functions:
        for blk in fn.blocks:
            for i in blk.instructions:
                if type(i).__name__ == "InstMatmult":
                    si = i.sync_info
                    for u in si.on_update if si else []:
                        if u.update_mode == "sem-inc" and u.update_reg is None:
                            pe_ids.add(u.id)
    if len(pe_ids) != 1:
        return 0
    (pe_id,) = pe_ids

    stripped = 0
    for fn in nc.m.functions:
        # thresholds observed anywhere in this function
        needed = set()
        for blk in fn.blocks:
            for i in blk.instructions:
                si = i.sync_info
                for w in si.on_wait if si else []:
                    if w.id == pe_id and w.wait_mode == "sem-ge-imm":
                        needed.add(w.wait_value)

        # exactly one block may contain the MM update stream
        blocks_with = []
        for blk in fn.blocks:
            if any(
                (i.sync_info and any(u.id == pe_id for u in i.sync_info.on_update))
                for i in blk.instructions
            ):
                blocks_with.append(blk)
        if len(blocks_with) != 1:
            continue
        for blk in blocks_with:
            insts = list(blk.instructions)
            mm_updates = []  # (inst, cum_count_after)
            cum = 0
            ok = True
            for i in insts:
                si = i.sync_info
                ups = [
                    u
                    for u in (si.on_update if si else [])
                    if u.id == pe_id
                ]
                if not ups:
                    continue
                if (
                    type(i).__name__ != "InstMatmult"
                    or len(ups) != 1
                    or ups[0].update_mode != "sem-inc"
                    or ups[0].update_reg is not None
                    or ups[0].update_value != 1
                ):
                    ok = False  # unexpected updater: leave block alone
                    break
                cum += 1
                mm_updates.append((i, cum))
            if not ok or not mm_updates:
                continue
            total = cum
            kept_cums = sorted(
                {c for (_, c) in mm_updates if c in needed} | {total}
            )
            if len(kept_cums) == len(mm_updates):
                continue
            kept_set = set(kept_cums)
            for inst, c in mm_updates:
                if c not in kept_set:
                    si = inst.sync_info
                    inst.sync_info = mybir.SyncInfo(
                        on_wait=list(si.on_wait), on_update=[]
                    )
                    stripped += 1
            # remap thresholds in this block (body waits) — other blocks
            # (reset/exit) reference `total`, remapped below via needed map
            import bisect

            def remap(k):
                return bisect.bisect_right(kept_cums, k) if k > 0 else 0

            for b2 in fn.blocks:
                for i in b2.instructions:
                    si = i.sync_info
                    if si is None or not si.on_wait:
                        continue
                    ws = list(si.on_wait)
                    changed = False
                    nws = []
                    for w in ws:
                        if w.id == pe_id and w.wait_mode == "sem-ge-imm":
                            nk = remap(w.wait_value)
                            if nk != w.wait_value:
                                w = mybir.SyncWait(
                                    sync_type=w.sync_type, id=w.id,
                                    ant_name=w.ant_name,
                                    wait_mode=w.wait_mode, wait_value=nk,
                                )
                                changed = True
                        nws.append(w)
                    if changed:
                        i.sync_info = mybir.SyncInfo(
                            on_wait=nws, on_update=list(si.on_update)
                        )
    return stripped


def build_program(repeat: int = 1, hw_loop: int = 0, ablate: str = "none",
                  unroll: int = 1, strip_sem: bool = False):
    nc = bacc.Bacc("TRN2", target_bir_lowering=False, debug=False)

    x_d = nc.dram_tensor(
        "x", [NG, 128, G, ROWS], BF16, kind="ExternalInput"
    ).ap()
    w1_d = nc.dram_tensor(
        "W1", [NG, 128, G, H1], BF16, kind="ExternalInput"
    ).ap()
    w2_d = nc.dram_tensor(
        "W2", [128, H1 // 128, H2], BF16, kind="ExternalInput"
    ).ap()
    w3_d = nc.dram_tensor("W3", [K3, N_PAD], BF16, kind="ExternalInput").ap()
    b1_d = nc.dram_tensor("b1", [128, H1 // 128], F32, kind="ExternalInput").ap()
    b2_d = nc.dram_tensor("b2", [H2, 1], F32, kind="ExternalInput").ap()
    out_d = nc.dram_tensor("out", [ROWS, N_CLS], F32, kind="ExternalOutput").ap()

    with tile.TileContext(nc) as tc, ExitStack() as ctx:
        const = ctx.enter_context(tc.tile_pool(name="const", bufs=1))
        xt_p = ctx.enter_context(tc.tile_pool(name="xt", bufs=XBUFS))
        h_p = ctx.enter_context(tc.tile_pool(name="h", bufs=4))
        o_p = ctx.enter_context(tc.tile_pool(name="o", bufs=2))
        ph1_p = ctx.enter_context(tc.tile_pool(name="ph1", bufs=4, space="PSUM"))
        ph2_p = ctx.enter_context(tc.tile_pool(name="ph2", bufs=2, space="PSUM"))
        po_p = ctx.enter_context(tc.tile_pool(name="po", bufs=2, space="PSUM"))

        # ---- weights/constants: prepacked on host ----
        # W1 is group-major; in the single-pass (graded) build its group
        # chunks are interleaved with the x stream inside body() so the PE
        # starts ~4us in instead of waiting 9us for the whole W1. Small
        # constants go on the scalar queue, off the stream queue.
        # one tile per W1 group so matmul ki depends on exactly its own
        # group's DMA (a single big tile would make the first matmul wait
        # on every W1 write if tracking is tile-granular)
        w1g = [
            const.tile([128, G, H1], BF16, name=f"w1g_{g}") for g in range(NG)
        ]
        if hw_loop > 0:
            for g in range(NG):
                nc.gpsimd.dma_start(w1g[g][:], w1_d[g])
        w2_sb = const.tile([128, H1 // 128, H2], BF16)
        nc.scalar.dma_start(w2_sb[:], w2_d)
        w3x_sb = const.tile([K3, N_PAD], BF16)
        nc.scalar.dma_start(w3x_sb[:], w3_d)
        b1_sb = const.tile([128, H1 // 128], F32)
        nc.scalar.dma_start(b1_sb[:], b1_d)
        b2_sb = const.tile([H2, 1], F32)
        nc.scalar.dma_start(b2_sb[:], b2_d)

        # persistent h2^T slots: rows 64:96 (ones row + zeros) never change,
        # so initialize them once; per-iteration ACT rewrites only rows 0:64
        h2t_slots = []
        for i in range(N_BLK):
            t = const.tile([K3, BLK], BF16, name=f"h2ts_{i}")
            nc.gpsimd.memset(t[H2:K3, :], 0.0)
            nc.gpsimd.memset(t[H2 : H2 + 1, :], 1.0)
            h2t_slots.append(t)

        xt_c = None
        if ablate == "l1only":
            # pure PE throughput probe: x resident in SBUF, loop is only
            # the 200 L1 matmuls
            xt_c = [const.tile([128, G, ROWS], BF16, name=f"xtc_{g}") for g in range(NG)]
            for g in range(NG):
                nc.sync.dma_start(xt_c[g][:], x_d[g])

        def body_l1only(it):
            ph1 = [
                [
                    ph1_p.tile([128, BLK], F32, tag="ph1", bufs=4,
                               name=f"ph1_{it}_{oi}_{blk}")
                    for blk in range(N_BLK)
                ]
                for oi in range(H1 // 128)
            ]
            for ki in range(KI):
                g, l = divmod(ki, G)
                for oi in range(H1 // 128):
                    for blk in range(N_BLK):
                        nc.tensor.matmul(
                            ph1[oi][blk][:],
                            w1g[g][:, l, oi * 128 : (oi + 1) * 128],
                            xt_c[g][:, l, blk * BLK : (blk + 1) * BLK],
                            start=(ki == 0),
                            stop=(ki == KI - 1),
                        )

        def body(it):
            if ablate == "empty":
                return
            # stream x^T: NG group DMAs, each [128, G, 1024] bf16; in the
            # single-pass build W1 group chunks ride the same queue in ki
            # order so matmul ki is never waiting on far-away weights
            xt = []
            for g in range(NG):
                if hw_loop == 0 and it == 0:
                    nc.gpsimd.dma_start(w1g[g][:], w1_d[g])
                t = xt_p.tile(
                    [128, G, ROWS], BF16, tag="xt", bufs=XBUFS,
                    name=f"xt_{it}_{g}",
                )
                nc.gpsimd.dma_start(t[:], x_d[g])
                xt.append(t)
            if ablate == "dma":
                return

            # L1: accumulate h1^T over 50 k-chunks into 4 PSUM banks.
            # blk-major order on the last ki lets blk0's ACT start while
            # PE finishes blk1's accumulation.
            ph1 = [
                [
                    ph1_p.tile(
                        [128, BLK], F32, tag="ph1", bufs=4,
                        name=f"ph1_{it}_{oi}_{blk}",
                    )
                    for blk in range(N_BLK)
                ]
                for oi in range(H1 // 128)
            ]
            n_oi = 1 if ablate == "dma_l1_half" else H1 // 128
            for ki in range(KI):
                g, l = divmod(ki, G)
                last = ki == KI - 1
                order = (
                    [(oi, blk) for blk in range(N_BLK) for oi in range(n_oi)]
                    if last
                    else [(oi, blk) for oi in range(n_oi) for blk in range(N_BLK)]
                )
                for oi, blk in order:
                    nc.tensor.matmul(
                        ph1[oi][blk][:],
                        w1g[g][:, l, oi * 128 : (oi + 1) * 128],
                        xt[g][:, l, blk * BLK : (blk + 1) * BLK],
                        start=(ki == 0),
                        stop=last,
                    )

            if ablate in ("dma_l1", "dma_l1_half"):
                return
            # tail stages interleaved across blocks so ACT drains all h1
            # evacuations back-to-back while PE runs the L2s — the terminal
            # (blk1) chain isn't stuck behind blk0's h2 on the ACT queue
            h1ts = []
            for blk in range(N_BLK):
                h1t = []
                for oi in range(H1 // 128):
                    ht = h_p.tile(
                        [128, BLK], BF16, tag="h1t", bufs=4,
                        name=f"h1t_{it}_{blk}_{oi}",
                    )
                    nc.scalar.activation(
                        ht[:],
                        ph1[oi][blk][:],
                        RELU,
                        bias=b1_sb[:, oi : oi + 1],
                    )
                    h1t.append(ht)
                h1ts.append(h1t)

            ph2s = []
            for blk in range(N_BLK):
                ph2 = ph2_p.tile(
                    [H2, BLK], F32, tag="ph2", bufs=2, name=f"ph2_{it}_{blk}"
                )
                for ci in range(H1 // 128):
                    nc.tensor.matmul(
                        ph2[:],
                        w2_sb[:, ci, :],
                        h1ts[blk][ci][:],
                        start=(ci == 0),
                        stop=(ci == H1 // 128 - 1),
                    )
                ph2s.append(ph2)

            for blk in range(N_BLK):
                nc.scalar.activation(
                    h2t_slots[blk][:H2, :], ph2s[blk][:], RELU, bias=b2_sb[:]
                )

            pos = []
            for blk in range(N_BLK):
                po = po_p.tile(
                    [128, RSUB * N_PAD], F32, tag="po", bufs=2,
                    name=f"po_{it}_{blk}",
                )
                for rs in range(RSUB):
                    nc.tensor.matmul(
                        po[:, rs * N_PAD : (rs + 1) * N_PAD],
                        h2t_slots[blk][:, rs * 128 : (rs + 1) * 128],
                        w3x_sb[:],
                        start=True,
                        stop=True,
                    )
                pos.append(po)

            for blk in range(N_BLK):
                ot = o_p.tile(
                    [128, RSUB * N_CLS], F32, tag="ot", bufs=2,
                    name=f"ot_{it}_{blk}",
                )
                nc.vector.tensor_copy(
                    ot[:].rearrange("p (rs c) -> p rs c", c=N_CLS),
                    pos[blk][:].rearrange("p (rs c) -> p rs c", c=N_PAD)[
                        :, :, :N_CLS
                    ],
                )
                nc.sync.dma_start(
                    out_d[blk * BLK : (blk + 1) * BLK, :].rearrange(
                        "(rs p) c -> p rs c", p=128
                    ),
                    ot[:].rearrange("p (rs c) -> p rs c", c=N_CLS),
                )

        body_fn = body_l1only if ablate == "l1only" else body
        if hw_loop > 0:
            # The For_i loop boundary is a barrier (the Tile framework emits
            # semaphore resets + InstDrain per trip), so nothing pipelines
            # across it: the first x-group DMA and the whole L2/L3/out tail
            # are exposed every trip (~5-9us/iter). Unrolling the body
            # amortizes the barrier and lets iterations pipeline within it.
            with tc.For_i(0, hw_loop):
                for u in range(unroll):
                    body_fn(u)
        else:
            for it in range(repeat):
                body_fn(it)

    nc.compile()
    if strip_sem:
        strip_mm_updates(nc)
    return nc


def make_in_maps(inputs):
    x = np.ascontiguousarray(inputs["x"], dtype=np.float32).reshape(
        ROWS_TOTAL, D_IN
    )
    xb = x.astype(BF)

    W1 = np.asarray(inputs["W1"], dtype=np.float32)
    w1p = np.ascontiguousarray(
        W1.reshape(NG, G, 128, H1).transpose(0, 2, 1, 3)
    ).astype(BF)
    W2 = np.asarray(inputs["W2"], dtype=np.float32)
    w2p = np.ascontiguousarray(
        W2.reshape(H1 // 128, 128, H2).transpose(1, 0, 2)
    ).astype(BF)
    w3x = np.zeros((K3, N_PAD), dtype=np.float32)
    w3x[:H2, :N_CLS] = np.asarray(inputs["W3"], dtype=np.float32)
    w3x[H2, :N_CLS] = np.asarray(inputs["b3"], dtype=np.float32)
    b1p = np.ascontiguousarray(
        np.asarray(inputs["b1"], dtype=np.float32).reshape(H1 // 128, 128).T
    )
    b2p = np.asarray(inputs["b2"], dtype=np.float32).reshape(H2, 1)
    common = {
        "W1": w1p,
        "W2": w2p,
        "W3": w3x.astype(BF),
        "b1": b1p,
        "b2": b2p,
    }
    in_maps = []
    for c in range(N_CORES):
        xc = xb[c * ROWS : (c + 1) * ROWS].view(np.uint16)
        xt = np.ascontiguousarray(
            xc.reshape(ROWS, NG, G, 128).transpose(1, 3, 2, 0)
        ).view(BF)
        in_maps.append({"x": xt, **common})
    return in_maps


_NC_CACHE = None


def kernel(**inputs) -> np.ndarray:
    global _NC_CACHE
    if _NC_CACHE is None:
        _NC_CACHE = build_program()
    nc = _NC_CACHE

    in_maps = make_in_maps(inputs)
    res = run_bass_kernel_spmd(nc, in_maps, list(range(N_CORES)))
    out = np.concatenate([res.results[i]["out"] for i in range(N_CORES)], axis=0)
    return out.reshape(16, 512, N_CLS).astype(np.float32)

